# revision 1
# baseline (speedup 1.0000x reference)
"""Trainium2 Bass kernel for nn_Depth_CA (depth-coded-aperture Wiener pipeline).

Strategy
--------
Every fft/ifft+shift combo in the reference is a constant 256x256 complex
matrix sandwich Y = A @ X @ A.T (all four transform matrices F@P, P@G,
P@F@P, P@G@P satisfy B == A.T).  On the PE array each sandwich is two
matmul groups with the DATA as the stationary operand and host-precomputed
constants [ATr|ATi], [-ATi|ATr] as 512-wide moving operands; PSUM
accumulation implements the complex arithmetic, so no transposes and no
vector-engine combine work are needed:

    MM1: PSUM = X^T @ A^T   (= (A X)^T)          X      stationary
    MM2: PSUM = (A X) @ A^T (= A X A^T)          (AX)^T stationary

Matmuls run in float32r (TF32-class, ~1.4e-4/MM, full PE rate at free
dim >= 256); rounding to f32r rides on the PSUM-drain copies.

Sharding: depths padded 15->16, 2 per core across 8 cores.  Each core
computes its 6 (depth, band) PSF units, the 12 image/result FFTs
(replicated; imgft interleaved into stage 1 for scheduler slack), the
blur and Wiener stages for its own depths, with per-batch AllReduce(add)
collectives (overlapped with blur compute) for the depth-summed `result`
and one AllReduce(max) for the final normalization.  The mid-pipeline
result/max(result) provably cancels and is skipped.  The blur/Wiener
inverse transforms are mathematically real, so their second matmul
groups compute only the real part and magnitude is a single Abs.

Measured on 8 axon-tunneled trn2 cores: ~775 us HW exec (NTFF),
relative error 4.8e-4 vs the fp32 jax reference.
"""
import os
import sys

for _p in ("/opt/trn_rl_repo", os.path.expanduser("~/.axon_site/_ro/trn_rl_repo")):
    if os.path.isdir(_p) and _p not in sys.path:
        sys.path.insert(0, _p)

import numpy as np

N = 256
ND, NB, B = 15, 3, 4
NDP = 16               # padded depth count
NCORES = 8
DPC = NDP // NCORES    # depths per core = 2

# ---------------------------------------------------------------- host constants
def _host_constants():
    ZI, Z0, RADII, PX = 0.05, 2.5, 0.002, 6.22e-6
    F_ = 1.0 / (1.0 / ZI + 1.0 / Z0)
    L_SEN = PX * N
    L_LEN = 2 * RADII * 2
    LAMB = np.array([460.0, 550.0, 640.0]) * 1e-9

    def deta(l_um):
        l = np.asarray(l_um, dtype=np.float64)
        return (1.5375 + 0.00829045 * l**-2 - 0.000211046 * l**-4) - 1.0

    R_ = F_ * deta(5.5e-7 * 1e6)
    FLMB = R_ / deta(LAMB * 1e6)
    ZS = np.sort(-3 * np.log(np.linspace(0.9, 11, ND)) + 8)
    DU = L_LEN / N
    u = np.arange(-L_LEN / 2, L_LEN / 2, DU)
    X_, Y_ = np.meshgrid(u, u)
    XY = X_ * X_ + Y_ * Y_
    RAD = (np.sqrt(XY) <= RADII).astype(np.float64)
    fx1 = np.fft.fftshift(np.arange(-1 / (2 * DU), 1 / (2 * DU), 1 / L_LEN))
    FX1, FY1 = np.meshgrid(fx1, fx1)
    FF = FX1 * FX1 + FY1 * FY1

    K_ = 2 * np.pi / LAMB
    COEF = (-K_ / (2 * FLMB[0]))[None, :] + K_[None, :] / (2 * ZS[:, None]) \
        + (np.pi * (L_LEN - L_SEN) / (LAMB * ZI * L_LEN))[None, :]
    PHASE1 = (COEF[:, :, None, None] * XY[None, None]).astype(np.float32)
    PHASE2 = ((np.pi * LAMB * ZI * L_LEN / L_SEN)[None, :, None, None]
              * FF[None, None]).astype(np.float32)

    W1 = RAD[None, None] * np.exp(1j * PHASE1.astype(np.float64))    # (15,3,N,N)
    W2 = np.exp(-1j * PHASE2.astype(np.float64)[0])                  # (3,N,N)

    j = np.arange(N)
    F = np.exp(-2j * np.pi * np.outer(j, j) / N)
    G = np.conj(F) / N
    P = np.zeros((N, N))
    P[j, (j + N // 2) % N] = 1.0
    A1 = F @ P
    A2 = P @ G
    Fc = P @ F @ P
    Gc = P @ G @ P
    return W1, W2, (A1, A2, Fc, Gc)


def _pack_field(X):
    """complex (N,N) -> float32 [2, 128, 512] = per row-block [Re | Im]."""
    out = np.empty((2, 128, 512), np.float32)
    for rb in range(2):
        out[rb, :, 0:256] = X.real[rb * 128:(rb + 1) * 128, :]
        out[rb, :, 256:512] = X.imag[rb * 128:(rb + 1) * 128, :]
    return out


def _pack_moving(A):
    """constant A -> float32 [2 variants, 2 k-chunks, 128, 512] moving ops."""
    AT = A.T.copy()
    out = np.empty((2, 2, 128, 512), np.float32)
    for k in range(2):
        r = AT.real[k * 128:(k + 1) * 128, :]
        i = AT.imag[k * 128:(k + 1) * 128, :]
        out[0, k, :, 0:256] = r
        out[0, k, :, 256:512] = i
        out[1, k, :, 0:256] = -i
        out[1, k, :, 256:512] = r
    return out


_CONST_CACHE = {}


def _get_device_arrays():
    """Host constants packed into the device DMA layouts."""
    if "dev" not in _CONST_CACHE:
        W1, W2, mats = _host_constants()
        # moving constants [128, 8192]: col = ((a*2+v)*2+k)*512 + n
        movA = np.concatenate(
            [_pack_moving(A).reshape(4, 128, 512).transpose(1, 0, 2).reshape(128, 2048)
             for A in mats], axis=1)
        # w2 [128, 3072]: col = (c*2+rb)*512 + n  (per-rb [Re|Im])
        w2p = np.concatenate(
            [_pack_field(W2[c]).transpose(1, 0, 2).reshape(128, 1024)
             for c in range(NB)], axis=1)
        # w1 table [48, 128, 1024] d-major over padded depths
        w1rows = []
        for d in range(NDP):
            dd = d if d < ND else 0
            for c in range(NB):
                w1rows.append(_pack_field(W1[dd, c]).transpose(1, 0, 2).reshape(128, 1024))
        w1all = np.stack(w1rows)
        R = np.kron(np.eye(16), np.ones((1, 16))).astype(np.float32)
        _CONST_CACHE["dev"] = (np.ascontiguousarray(movA), np.ascontiguousarray(w2p),
                               np.ascontiguousarray(w1all), R)
    return _CONST_CACHE["dev"]


# ---------------------------------------------------------------- device program
_REPS = int(os.environ.get("BASS_KERNEL_REPS", "1"))

A1_I, A2_I, FC_I, GC_I = 0, 1, 2, 3


def _build_program():
    host_arrays = _get_device_arrays()
    reps = _REPS
    import concourse.bass as bass
    import concourse.bass_isa as bass_isa
    import concourse.bacc as bacc
    import concourse.mybir as mybir
    import concourse.tile as tile
    import base64, io

    dt = mybir.dt
    ALU = mybir.AluOpType
    ACTF = mybir.ActivationFunctionType

    movA_h, w2_h, w1all_h, R_h = host_arrays

    nc = bacc.Bacc("TRN2", target_bir_lowering=False, debug=False,
                   num_devices=NCORES)

    def inline(data, name, f32r=False):
        h = nc.inline_tensor(np.ascontiguousarray(data), name=name)
        if f32r:
            mls = nc.lookup_mls(h)
            mls.dtype = dt.float32r
            h = bass.DRamTensorHandle(name, list(data.shape), dt.float32r)
        return h.ap()

    movA_d = inline(movA_h, "mova", f32r=True)                 # [128, 8192]
    w2_d = inline(w2_h, "w2")                                  # [128, 3072]
    w1all_d = inline(w1all_h, "w1all")                         # [48, 128, 1024]
    r_d = inline(R_h, "rmat")                                  # [16, 256]

    img_d = nc.dram_tensor("imgf", [128, 6144], dt.float32r, kind="ExternalInput").ap()
    map_d = nc.dram_tensor("mapf", [B, 128, DPC * 512], dt.float32, kind="ExternalInput").ap()
    ht_d = nc.dram_tensor("ht", [16, 16], dt.float32, kind="ExternalInput").ap()
    par_d = nc.dram_tensor("param", [1, 1], dt.float32, kind="ExternalInput").ap()
    mask_d = nc.dram_tensor("mask", [1, DPC], dt.float32, kind="ExternalInput").ap()
    out_d = nc.dram_tensor("out_recov", [DPC, NB, B, 128, 512], dt.float32, kind="ExternalOutput").ap()

    with tile.TileContext(nc) as tc:
        with (
            tc.tile_pool(name="res", bufs=1) as res,
            tc.tile_pool(name="wk", bufs=2) as wk,
            tc.tile_pool(name="ps", bufs=4, space="PSUM") as ps,
            tc.tile_pool(name="dram", bufs=1, space="DRAM") as dram,
        ):
            # ---------------- resident constants (single DMAs)
            movall = res.tile([128, 8192], dt.float32r, tag="movall", name="movall")
            nc.sync.dma_start(movall[:], movA_d[:])

            def mov(a, v, k):
                o = ((a * 2 + v) * 2 + k) * 512
                return movall[:, o:o + 512]


            w2all = res.tile([128, 3072], dt.float32, tag="w2all", name="w2all")
            nc.sync.dma_start(w2all[:], w2_d[:])

            def w2v(c, rb):
                o = (c * 2 + rb) * 512
                return w2all[:, o:o + 512]

            par1 = res.tile([1, 1], dt.float32, tag="par1", name="par1")
            nc.sync.dma_start(par1[:], par_d[:])
            par128 = res.tile([128, 1], dt.float32, tag="par128", name="par128")
            nc.gpsimd.partition_broadcast(par128[:], par1[:])
            mask1 = res.tile([1, DPC], dt.float32, tag="mask1", name="mask1")
            nc.sync.dma_start(mask1[:], mask_d[:])
            mask128 = res.tile([128, DPC], dt.float32, tag="mask128", name="mask128")
            nc.gpsimd.partition_broadcast(mask128[:], mask1[:])

            # ---------------- CA = R^T @ (H @ R)  (plain fp32)
            ht_t = res.tile([16, 16], dt.float32, tag="ht_t", name="ht_t")
            r_t = res.tile([16, 256], dt.float32, tag="r_t", name="r_t")
            nc.sync.dma_start(ht_t[:], ht_d[:])
            nc.sync.dma_start(r_t[:], r_d[:])
            ca_mid_ps = ps.tile([16, 256], dt.float32, tag="psB", bufs=4, name="ca_mid_ps")
            nc.tensor.matmul(ca_mid_ps[:], ht_t[:], r_t[:], start=True, stop=True)
            ca_mid = res.tile([16, 256], dt.float32, tag="ca_mid", name="ca_mid")
            nc.vector.tensor_copy(ca_mid[:], ca_mid_ps[:])
            ca = [res.tile([128, 256], dt.float32, tag=f"ca{mb}", name=f"ca{mb}")
                  for mb in range(2)]
            for mb in range(2):
                ca_ps = ps.tile([128, 256], dt.float32, tag="psB", bufs=4, name=f"ca_ps{mb}")
                nc.tensor.matmul(ca_ps[:], r_t[:, mb * 128:(mb + 1) * 128],
                                 ca_mid[:], start=True, stop=True)
                nc.vector.tensor_copy(ca[mb][:], ca_ps[:])

            # ---------------- helpers
            MM1_NAMES = ("s1a", "s1c", "pfa", "pia", "ifa", "bla", "rfa", "wna")

            def mm_sandwich_half(stat, a_idx, is_complex, name):
                """PSUM[mb] = S^T @ A^T.  `stat` = list of 2 per-k-chunk APs:
                complex: [128,512] ([Re|Im]); real: [128,256]."""
                ptag = "psA" if name in MM1_NAMES else "psB"
                psums = []
                for mb in range(2):
                    acc = ps.tile([128, 512], dt.float32, tag=ptag, bufs=4, name=f"{name}_ps{mb}")
                    mms = []
                    for k in range(2):
                        mms.append((stat[k][:, mb * 128:(mb + 1) * 128], mov(a_idx, 0, k)))
                        if is_complex:
                            mms.append((stat[k][:, 256 + mb * 128:256 + (mb + 1) * 128],
                                        mov(a_idx, 1, k)))
                    for i, (lhsT, rhs) in enumerate(mms):
                        nc.tensor.matmul(acc[:], lhsT, rhs,
                                         start=(i == 0), stop=(i == len(mms) - 1))
                    psums.append(acc)
                return psums

            def mm_sandwich_real_out(stat, a_idx, name):
                """Re-only PSUM[mb][128,256] = Re(S^T @ A^T), S complex packed."""
                psums = []
                for mb in range(2):
                    acc = ps.tile([128, 256], dt.float32, tag="psB", bufs=4, name=f"{name}_ps{mb}")
                    mms = []
                    for k in range(2):
                        mms.append((stat[k][:, mb * 128:(mb + 1) * 128],
                                    mov(a_idx, 0, k)[:, 0:256]))
                        mms.append((stat[k][:, 256 + mb * 128:256 + (mb + 1) * 128],
                                    mov(a_idx, 1, k)[:, 0:256]))
                    for i, (lhsT, rhs) in enumerate(mms):
                        nc.tensor.matmul(acc[:], lhsT, rhs,
                                         start=(i == 0), stop=(i == len(mms) - 1))
                    psums.append(acc)
                return psums

            def drain_f32r(psums, name):
                dtag, dbufs = ("drB", 6) if name in ("blu", "wnu") else ("drA", 8)
                out = [wk.tile([128, 512], dt.float32r, tag=dtag, bufs=dbufs, name=f"{name}{mb}")
                       for mb in range(2)]
                nc.scalar.copy(out[0][:], psums[0][:])
                nc.vector.tensor_copy(out[1][:], psums[1][:])
                return out

            def cmul(out_rb, x_rb, y_rb, pool_ok=False):
                """one-rb complex mult: out [128,512] = x * y ([Re|Im] packed).
                pool_ok: inputs are SBUF-only -> run 2 of 6 ops on idle GpSimd."""
                xr, xi = x_rb[:, 0:256], x_rb[:, 256:512]
                yr, yi = y_rb[:, 0:256], y_rb[:, 256:512]
                eng2 = nc.vector
                t1 = wk.tile([128, 256], dt.float32, tag="cms", bufs=12, name="cmt1")
                t2 = wk.tile([128, 256], dt.float32, tag="cms", bufs=12, name="cmt2")
                t3 = wk.tile([128, 256], dt.float32, tag="cms", bufs=12, name="cmt3")
                t4 = wk.tile([128, 256], dt.float32, tag="cms", bufs=12, name="cmt4")
                nc.vector.tensor_tensor(t1[:], xr, yr, op=ALU.mult)
                eng2.tensor_tensor(t2[:], xi, yi, op=ALU.mult)
                nc.vector.tensor_tensor(out_rb[:, 0:256], t1[:], t2[:], op=ALU.subtract)
                nc.vector.tensor_tensor(t3[:], xr, yi, op=ALU.mult)
                eng2.tensor_tensor(t4[:], xi, yr, op=ALU.mult)
                nc.vector.tensor_tensor(out_rb[:, 256:512], t3[:], t4[:], op=ALU.add)

            # ---------------- resident per-unit products
            psffr_t = [res.tile([128, 512], dt.float32, tag=f"psffr{i}", name=f"psffr{i}")
                       for i in range(DPC * NB * 2)]
            runmax = [res.tile([128, 1], dt.float32, tag=f"runmax{dl}", name=f"runmax{dl}")
                      for dl in range(DPC)]

            imgft_dr = dram.tile([B * NB, 128, 1024], dt.float32, name="imgft_dr")
            mag2_dr = dram.tile([DPC * NB * B, 128, 512], dt.float32, name="mag2_dr")
            kker_dr = dram.tile([DPC * NB, 128, 1024], dt.float32, name="kker_dr")

            pid6 = nc.gpsimd.partition_id() * (DPC * NB)

            def emit_imgft(f):
                imS = wk.tile([128, 512], dt.float32r, tag="imS", name="imS")
                nc.sync.dma_start(imS[:], img_d[:, f * 512:(f + 1) * 512])
                stat = [imS[:, 0:256], imS[:, 256:512]]
                iu1 = drain_f32r(mm_sandwich_half(stat, FC_I, False, "ifa"), "ifu")
                ip2 = mm_sandwich_half(iu1, FC_I, True, "ifb")
                imo = wk.tile([128, 1024], dt.float32, tag="cfld", bufs=3, name="imo")
                nc.scalar.copy(imo[:, 0:512], ip2[0][:])
                nc.vector.tensor_copy(imo[:, 512:1024], ip2[1][:])
                nc.scalar.dma_start(imgft_dr[f], imo[:])

            for _rep in range(reps):
                cc_in = [dram.tile([NB, 128, 512], dt.float32, name=f"cc_in{b}_r{_rep}")
                         for b in range(B)]
                cc_out = [dram.tile([NB, 128, 512], dt.float32, name=f"cc_out{b}_r{_rep}",
                                    addr_space="Shared") for b in range(B)]
                ccm_in = dram.tile([1, 16], dt.float32, name=f"ccm_in_r{_rep}")
                ccm_out = dram.tile([1, 16], dt.float32, name=f"ccm_out_r{_rep}", addr_space="Shared")
                # ======== stage 1: psf, psffr, K per (dloc, c),
                # interleaved with imgft units for scheduler slack
                for u in range(DPC * NB):
                    for f in (2 * u, 2 * u + 1):
                        emit_imgft(f)
                    c = u % NB
                    w1t = wk.tile([128, 1024], dt.float32, tag="w1t", name="w1t")
                    nc.gpsimd.dma_start(w1t[:], w1all_d[bass.ds(pid6 + u, 1)])
                    ph = wk.tile([128, 1024], dt.float32r, tag="ph", name="ph")
                    for rb in range(2):
                        o = rb * 512
                        nc.vector.tensor_tensor(ph[:, o:o + 256], w1t[:, o:o + 256],
                                                ca[rb][:], op=ALU.mult)
                        nc.vector.tensor_tensor(ph[:, o + 256:o + 512], w1t[:, o + 256:o + 512],
                                                ca[rb][:], op=ALU.mult)
                    phs = [ph[:, 0:512], ph[:, 512:1024]]
                    u1 = drain_f32r(mm_sandwich_half(phs, A1_I, True, "s1a"), "s1u1")
                    ps2 = mm_sandwich_half(u1, A1_I, True, "s1b")
                    vu2 = wk.tile([128, 1024], dt.float32r, tag="cprod", bufs=3, name="vu2")
                    for rb in range(2):
                        cmul(vu2[:, rb * 512:(rb + 1) * 512], ps2[rb], w2v(c, rb))
                    vus = [vu2[:, 0:512], vu2[:, 512:1024]]
                    u3 = drain_f32r(mm_sandwich_half(vus, A2_I, True, "s1c"), "s1u3")
                    ps4 = mm_sandwich_half(u3, A2_I, True, "s1d")
                    # psf = |vu3|^2 normalized (real field, rb-packed [128,512])
                    psfu = wk.tile([128, 512], dt.float32, tag="psfu", name="psfu")
                    for rb in range(2):
                        t1 = wk.tile([128, 256], dt.float32, tag="cms", bufs=12, name="sq1")
                        t2 = wk.tile([128, 256], dt.float32, tag="cms", bufs=12, name="sq2")
                        nc.scalar.activation(t1[:], ps4[rb][:, 0:256], ACTF.Square)
                        nc.scalar.activation(t2[:], ps4[rb][:, 256:512], ACTF.Square)
                        nc.vector.tensor_tensor(psfu[:, rb * 256:(rb + 1) * 256],
                                                t1[:], t2[:], op=ALU.add)
                    sums = wk.tile([128, 1], dt.float32, tag="sums", name="sums")
                    nc.vector.tensor_reduce(sums[:], psfu[:], axis=mybir.AxisListType.X, op=ALU.add)
                    tot128 = wk.tile([128, 1], dt.float32, tag="tot128", name="tot128")
                    nc.gpsimd.partition_all_reduce(tot128[:], sums[:], channels=128,
                                                   reduce_op=bass_isa.ReduceOp.add)
                    inv128 = wk.tile([128, 1], dt.float32, tag="inv128", name="inv128")
                    nc.vector.reciprocal(inv128[:], tot128[:])
                    psft = wk.tile([128, 512], dt.float32r, tag="psft", name="psft")
                    nc.vector.tensor_scalar_mul(psft[:], psfu[:], inv128[:])
                    psts = [psft[:, 0:256], psft[:, 256:512]]
                    # psffr = Fc psf Fc
                    pu1 = drain_f32r(mm_sandwich_half(psts, FC_I, False, "pfa"), "pfu")
                    pp2 = mm_sandwich_half(pu1, FC_I, True, "pfb")
                    for rb in range(2):
                        nc.scalar.copy(psffr_t[u * 2 + rb][:], pp2[rb][:])
                    # K = (Gc psf Gc) / (|psffr|^2 + param)
                    piu = drain_f32r(mm_sandwich_half(psts, GC_I, False, "pia"), "piu")
                    pi2 = mm_sandwich_half(piu, GC_I, True, "pib")
                    kk = wk.tile([128, 1024], dt.float32, tag="cfld", bufs=3, name="kk")
                    for rb in range(2):
                        fr = psffr_t[u * 2 + rb][:, 0:256]
                        fi = psffr_t[u * 2 + rb][:, 256:512]
                        t1 = wk.tile([128, 256], dt.float32, tag="cms", bufs=12, name="ab1")
                        t2 = wk.tile([128, 256], dt.float32, tag="cms", bufs=12, name="ab2")
                        nc.vector.tensor_tensor(t1[:], fr, fr, op=ALU.mult)
                        nc.vector.tensor_tensor(t2[:], fi, fi, op=ALU.mult)
                        nc.vector.tensor_tensor(t1[:], t1[:], t2[:], op=ALU.add)
                        nc.vector.tensor_scalar_add(t1[:], t1[:], par128[:])
                        invp = wk.tile([128, 256], dt.float32, tag="cms", bufs=12, name="invp")
                        nc.vector.reciprocal(invp[:], t1[:])
                        nc.vector.tensor_tensor(kk[:, rb * 512:rb * 512 + 256],
                                                pi2[rb][:, 0:256], invp[:], op=ALU.mult)
                        nc.vector.tensor_tensor(kk[:, rb * 512 + 256:rb * 512 + 512],
                                                pi2[rb][:, 256:512], invp[:], op=ALU.mult)
                    nc.scalar.dma_start(kker_dr[u], kk[:])

                # ======== blur stage: partial result per (b,c)
                for b in range(B):
                    mapt = wk.tile([128, DPC * 512], dt.float32, tag="mapt", bufs=2, name="mapt")
                    nc.sync.dma_start(mapt[:], map_d[b])
                    for c in range(NB):
                        f = b * NB + c
                        imf = wk.tile([128, 1024], dt.float32, tag="cfld", bufs=3, name="imf")
                        nc.scalar.dma_start(imf[:], imgft_dr[f])
                        racc = wk.tile([128, 512], dt.float32, tag="racc", name="racc")
                        nc.vector.memset(racc[:], 0.0)
                        for dl in range(DPC):
                            u = dl * NB + c
                            bp = wk.tile([128, 1024], dt.float32r, tag="cprod", bufs=3, name="bp")
                            for rb in range(2):
                                cmul(bp[:, rb * 512:(rb + 1) * 512],
                                     imf[:, rb * 512:(rb + 1) * 512], psffr_t[u * 2 + rb],
                                     pool_ok=True)
                            bps = [bp[:, 0:512], bp[:, 512:1024]]
                            bu1 = drain_f32r(mm_sandwich_half(bps, GC_I, True, "bla"), "blu")
                            bp2 = mm_sandwich_real_out(bu1, GC_I, "blb")
                            for rb in range(2):
                                mag = wk.tile([128, 256], dt.float32, tag="cms", bufs=12, name="mag")
                                nc.scalar.activation(mag[:], bp2[rb][:], ACTF.Abs)
                                t2 = wk.tile([128, 256], dt.float32, tag="cms", bufs=12, name="bm2")
                                nc.vector.tensor_tensor(
                                    t2[:], mag[:], mapt[:, (dl * 2 + rb) * 256:(dl * 2 + rb + 1) * 256],
                                    op=ALU.mult)
                                nc.vector.tensor_tensor(racc[:, rb * 256:(rb + 1) * 256],
                                                        racc[:, rb * 256:(rb + 1) * 256],
                                                        t2[:], op=ALU.add)
                        nc.sync.dma_start(cc_in[b][c], racc[:])
                    nc.gpsimd.collective_compute(
                        "AllReduce", ALU.add,
                        replica_groups=[list(range(NCORES))],
                        ins=[cc_in[b][:]], outs=[cc_out[b][:]],
                    )

                # ======== wiener stage
                for dl in range(DPC):
                    nc.vector.memset(runmax[dl][:], 0.0)
                for b in range(B):
                    for c in range(NB):
                        f = b * NB + c
                        rres = wk.tile([128, 512], dt.float32, tag="rres", name="rres")
                        nc.sync.dma_start(rres[:], cc_out[b][c])
                        res_t = wk.tile([128, 512], dt.float32r, tag="res_t", name="res_t")
                        nc.vector.tensor_copy(res_t[:], rres[:])
                        rsts = [res_t[:, 0:256], res_t[:, 256:512]]
                        ru1 = drain_f32r(mm_sandwich_half(rsts, FC_I, False, "rfa"), "rfu")
                        rp2 = mm_sandwich_half(ru1, FC_I, True, "rfb")
                        resfr = wk.tile([128, 1024], dt.float32, tag="cfld", bufs=3, name="resfr")
                        nc.scalar.copy(resfr[:, 0:512], rp2[0][:])
                        nc.vector.tensor_copy(resfr[:, 512:1024], rp2[1][:])
                        for dl in range(DPC):
                            u = dl * NB + c
                            kkt = wk.tile([128, 1024], dt.float32, tag="kkt", bufs=3, name="kkt")
                            nc.scalar.dma_start(kkt[:], kker_dr[u])
                            wn = wk.tile([128, 1024], dt.float32r, tag="cprod", bufs=3, name="wn")
                            for rb in range(2):
                                cmul(wn[:, rb * 512:(rb + 1) * 512],
                                     kkt[:, rb * 512:(rb + 1) * 512],
                                     resfr[:, rb * 512:(rb + 1) * 512], pool_ok=True)
                            wns = [wn[:, 0:512], wn[:, 512:1024]]
                            wu1 = drain_f32r(mm_sandwich_half(wns, GC_I, True, "wna"), "wnu")
                            wp2 = mm_sandwich_real_out(wu1, GC_I, "wnb")
                            mi = (dl * NB + c) * B + b
                            mag2 = wk.tile([128, 512], dt.float32, tag="mag2", bufs=2, name="mag2")
                            for rb in range(2):
                                nc.scalar.activation(mag2[:, rb * 256:(rb + 1) * 256],
                                                     wp2[rb][:], ACTF.Abs)
                            piece = wk.tile([128, 1], dt.float32, tag="piece", name="piece")
                            nc.vector.tensor_reduce(piece[:], mag2[:],
                                                    axis=mybir.AxisListType.X, op=ALU.max)
                            nc.vector.tensor_tensor(runmax[dl][:], runmax[dl][:], piece[:], op=ALU.max)
                            nc.scalar.dma_start(mag2_dr[mi], mag2[:])

                # ======== global max + final normalize
                mx = wk.tile([128, 1], dt.float32, tag="mx", name="mx")
                nc.vector.tensor_scalar_mul(mx[:], runmax[1][:], mask128[:, 1:2])
                nc.vector.tensor_tensor(mx[:], mx[:], runmax[0][:], op=ALU.max)
                gmx128 = wk.tile([128, 1], dt.float32, tag="gmx128", name="gmx128")
                nc.gpsimd.partition_all_reduce(gmx128[:], mx[:], channels=128,
                                               reduce_op=bass_isa.ReduceOp.max)
                ones16 = wk.tile([1, 16], dt.float32, tag="ones16", name="ones16")
                nc.vector.memset(ones16[:], 1.0)
                gmx16 = wk.tile([1, 16], dt.float32, tag="gmx16", name="gmx16")
                nc.vector.tensor_scalar_mul(gmx16[:], ones16[:], gmx128[0:1, :])
                nc.sync.dma_start(ccm_in[:], gmx16[:])
                nc.gpsimd.collective_compute(
                    "AllReduce", ALU.max,
                    replica_groups=[list(range(NCORES))],
                    ins=[ccm_in[:]], outs=[ccm_out[:]],
                )
                gm = wk.tile([1, 1], dt.float32, tag="gm", name="gm")
                nc.sync.dma_start(gm[:], ccm_out[0:1, 0:1])
                ginv = wk.tile([1, 1], dt.float32, tag="ginv", name="ginv")
                nc.vector.reciprocal(ginv[:], gm[:])
                ginv128 = wk.tile([128, 1], dt.float32, tag="ginv128", name="ginv128")
                nc.gpsimd.partition_broadcast(ginv128[:], ginv[:])

                for dl in range(DPC):
                    for c in range(NB):
                        for b in range(B):
                            mi = (dl * NB + c) * B + b
                            m2 = wk.tile([128, 512], dt.float32, tag="fin", bufs=3, name="m2")
                            nc.sync.dma_start(m2[:], mag2_dr[mi])
                            o = wk.tile([128, 512], dt.float32, tag="fin", bufs=3, name="o")
                            nc.scalar.activation(o[:], m2[:], ACTF.Copy, scale=ginv128[:])
                            nc.scalar.dma_start(out_d[dl, c, b], o[:])

    nc.compile()
    return nc


_PROG_CACHE = {}


def _get_program():
    if "nc" not in _PROG_CACHE:
        _PROG_CACHE["nc"] = _build_program()
    return _PROG_CACHE["nc"]


# ---------------------------------------------------------------- cached runner
def _make_runner():
    """Build the jitted SPMD callable once; reuse across kernel() calls."""
    import jax
    from jax.sharding import Mesh, PartitionSpec
    from jax.experimental.shard_map import shard_map
    import concourse.mybir as mybir
    from concourse import bass2jax

    bass2jax.install_neuronx_cc_hook()
    nc = _get_program()

    partition_name = nc.partition_id_tensor.name if nc.partition_id_tensor else None
    in_names, out_names, out_avals, zero_shapes = [], [], [], []
    for alloc in nc.m.functions[0].allocations:
        if not isinstance(alloc, mybir.MemoryLocationSet):
            continue
        if not alloc.memorylocations:
            continue
        name = alloc.memorylocations[0].name
        if alloc.kind == "ExternalInput":
            if name != partition_name:
                in_names.append(name)
        elif alloc.kind == "ExternalOutput":
            out_names.append(name)
            shape = tuple(alloc.tensor_shape)
            dtype = mybir.dt.np(alloc.dtype)
            out_avals.append(jax.core.ShapedArray(shape, dtype))
            zero_shapes.append((shape, dtype))
    n_params = len(in_names)
    n_outs = len(out_avals)
    all_in_names = list(in_names) + list(out_names)
    if partition_name is not None:
        all_in_names.append(partition_name)
    donate = tuple(range(n_params, n_params + n_outs))

    def _body(*args):
        operands = list(args)
        if partition_name is not None:
            operands.append(bass2jax.partition_id_tensor())
        outs = bass2jax._bass_exec_p.bind(
            *operands,
            out_avals=tuple(out_avals),
            in_names=tuple(all_in_names),
            out_names=tuple(out_names),
            lowering_input_output_aliases=(),
            sim_require_finite=True,
            sim_require_nnan=True,
            nc=nc,
        )
        return tuple(outs)

    devices = jax.devices()[:NCORES]
    mesh = Mesh(np.asarray(devices), ("core",))
    in_specs = (PartitionSpec("core"),) * (n_params + n_outs)
    out_specs = (PartitionSpec("core"),) * n_outs
    sharded = jax.jit(
        shard_map(_body, mesh=mesh, in_specs=in_specs, out_specs=out_specs,
                  check_rep=False),
        donate_argnums=donate, keep_unused=True)

    def run(in_maps):
        concat_in = [
            np.concatenate([np.asarray(m[name]) for m in in_maps], axis=0)
            for name in in_names
        ]
        concat_zeros = [
            np.zeros((NCORES * s[0], *s[1:]), d) for (s, d) in zero_shapes
        ]
        out_arrs = sharded(*concat_in, *concat_zeros)
        return [
            {name: np.asarray(out_arrs[i]).reshape(NCORES, *out_avals[i].shape)[c]
             for i, name in enumerate(out_names)}
            for c in range(NCORES)
        ]

    return run


def _get_runner():
    if "run" not in _PROG_CACHE:
        _PROG_CACHE["run"] = _make_runner()
    return _PROG_CACHE["run"]


# ---------------------------------------------------------------- entry point
def _build_in_maps(img, Map, H, parameter):
    # img fields (b,c) -> [128, 6144]: col = (f*2+k)*256 + x
    imgt = img.transpose(0, 3, 1, 2).reshape(B * NB, 2, 128, 256)
    imgf = np.ascontiguousarray(imgt.transpose(2, 0, 1, 3).reshape(128, B * NB * 512))
    # Map -> per-core [4, 128, DPC*512]: col = (dl*2+rb)*256 + x
    mapt = Map.transpose(3, 0, 1, 2).reshape(ND, B, 2, 128, 256)
    ht = np.ascontiguousarray(H.reshape(16, 16).T)
    par = parameter.reshape(1, 1)
    in_maps = []
    for core in range(NCORES):
        mp = np.zeros((B, 128, DPC * 512), np.float32)
        msk = np.zeros((1, DPC), np.float32)
        for dl in range(DPC):
            d = core * DPC + dl
            if d < ND:
                fld = mapt[d].transpose(0, 2, 1, 3).reshape(B, 128, 512)
                mp[:, :, dl * 512:(dl + 1) * 512] = fld
                msk[0, dl] = 1.0
        in_maps.append({
            "imgf": imgf, "mapf": mp, "ht": ht, "param": par, "mask": msk,
        })
    return in_maps


def kernel(img, Map, H, parameter):
    img = np.ascontiguousarray(np.asarray(img, np.float32))
    Map = np.ascontiguousarray(np.asarray(Map, np.float32))
    H = np.asarray(H, np.float32)
    parameter = np.asarray(parameter, np.float32)

    try:
        run = _get_runner()
    except Exception:
        run = None

    in_maps = _build_in_maps(img, Map, H, parameter)

    if run is not None:
        try:
            results = run(in_maps)
        except Exception:
            run = None
    if run is None:
        from concourse.bass_utils import run_bass_kernel_spmd
        rr = run_bass_kernel_spmd(_get_program(), in_maps,
                                  core_ids=list(range(NCORES)))
        results = rr.results

    out = np.empty((B, 256, 256, NB * ND), np.float32)
    for core in range(NCORES):
        rec = results[core]["out_recov"]            # [DPC, NB, B, 256, 256]
        for dl in range(DPC):
            d = core * DPC + dl
            if d >= ND:
                continue
            for c in range(NB):
                for b in range(B):
                    out[b, :, :, c * ND + d] = (
                        rec[dl, c, b].reshape(128, 2, 256)
                        .transpose(1, 0, 2).reshape(256, 256))
    return out



# revision 12
# speedup vs baseline: 1.0960x; 1.0960x over previous
"""Trainium2 Bass kernel for nn_Depth_CA (depth-coded-aperture Wiener pipeline).

Strategy (v2)
-------------
Every fft/ifft+shift combo in the reference is a constant 256x256 complex
matrix sandwich Y = A @ X @ A.T computed on the PE array in float32r with
the DATA stationary and host-precomputed constants [ATr|ATi], [-ATi|ATr]
as 512-wide moving operands; PSUM accumulation implements the complex
arithmetic.

v2 changes vs the ~758us baseline:
 * The elementwise exp(-i*PHASE2) multiply is rank-1 separable
   (W2 = outer(d,d)) and is folded into band-specific constants
   A2c = (P@G)@diag(d_c), removing a 6-op vector cmul from the psf
   critical path.
 * All three phases are software-pipelined: emission round-robins
   units through a generator pump so the in-order tensor queue always
   has another unit's matmuls between a sandwich stage and the PSUM
   drain it depends on (keeps the PE p-state ramped).
 * psf normalization is deferred: 1/sum rides the psffr PSUM-drain
   scale, and the Wiener denominator is scaled by sum instead.
 * The 2nd matmul group of the (mathematically real) blur/Wiener
   inverse transforms writes both row-blocks into one PSUM tile ->
   single Abs.
 * Complex multiplies are split across GpSimd(Pool)/DVE with rb-fused
   strided views; map-FMA and max-reduces run on Pool.
 * The final global max is taken per-core (no collective) and the
   division by max happens on host during unshard; outputs stream to
   HBM during the Wiener phase, killing the ~100us device tail.

Sharding: depths padded 15->16, 2 per core across 8 cores; per-batch
AllReduce(add) for the depth-summed `result` overlaps blur compute.
The mid-pipeline result/max(result) provably cancels and is skipped.
"""
import os
import sys

for _p in ("/opt/trn_rl_repo", os.path.expanduser("~/.axon_site/_ro/trn_rl_repo")):
    if os.path.isdir(_p) and _p not in sys.path:
        sys.path.insert(0, _p)

import numpy as np

N = 256
ND, NB, B = 15, 3, 4
NDP = 16               # padded depth count
NCORES = 8
DPC = NDP // NCORES    # depths per core = 2

# matrix slots in the packed moving-constant table
FC_I, A1_I, A2C_I, GC_I = 0, 1, 2, 5   # A2C_I + c for band c


# ---------------------------------------------------------------- host constants
def _host_constants():
    ZI, Z0, RADII, PX = 0.05, 2.5, 0.002, 6.22e-6
    F_ = 1.0 / (1.0 / ZI + 1.0 / Z0)
    L_SEN = PX * N
    L_LEN = 2 * RADII * 2
    LAMB = np.array([460.0, 550.0, 640.0]) * 1e-9

    def deta(l_um):
        l = np.asarray(l_um, dtype=np.float64)
        return (1.5375 + 0.00829045 * l**-2 - 0.000211046 * l**-4) - 1.0

    R_ = F_ * deta(5.5e-7 * 1e6)
    FLMB = R_ / deta(LAMB * 1e6)
    ZS = np.sort(-3 * np.log(np.linspace(0.9, 11, ND)) + 8)
    DU = L_LEN / N
    u = np.arange(-L_LEN / 2, L_LEN / 2, DU)
    X_, Y_ = np.meshgrid(u, u)
    XY = X_ * X_ + Y_ * Y_
    RAD = (np.sqrt(XY) <= RADII).astype(np.float64)
    fx1 = np.fft.fftshift(np.arange(-1 / (2 * DU), 1 / (2 * DU), 1 / L_LEN))

    K_ = 2 * np.pi / LAMB
    COEF = (-K_ / (2 * FLMB[0]))[None, :] + K_[None, :] / (2 * ZS[:, None]) \
        + (np.pi * (L_LEN - L_SEN) / (LAMB * ZI * L_LEN))[None, :]
    PHASE1 = (COEF[:, :, None, None] * XY[None, None]).astype(np.float32)
    W1 = RAD[None, None] * np.exp(1j * PHASE1.astype(np.float64))    # (15,3,N,N)

    j = np.arange(N)
    F = np.exp(-2j * np.pi * np.outer(j, j) / N)
    G = np.conj(F) / N
    P = np.zeros((N, N))
    P[j, (j + N // 2) % N] = 1.0
    A1 = F @ P
    A2 = P @ G
    Fc = P @ F @ P
    Gc = P @ G @ P
    # W2 = exp(-i*k2c*(fx_i^2+fx_j^2)) = outer(d_c, d_c) folded into A2
    mats = [Fc, A1]
    for c in range(NB):
        k2 = np.pi * LAMB[c] * ZI * L_LEN / L_SEN
        d = np.exp(-1j * k2 * fx1 ** 2)
        mats.append(A2 @ np.diag(d))
    mats.append(Gc)
    return W1, mats


def _pack_field(X):
    """complex (N,N) -> float32 [2, 128, 512] = per row-block [Re | Im]."""
    out = np.empty((2, 128, 512), np.float32)
    for rb in range(2):
        out[rb, :, 0:256] = X.real[rb * 128:(rb + 1) * 128, :]
        out[rb, :, 256:512] = X.imag[rb * 128:(rb + 1) * 128, :]
    return out


def _pack_moving(A):
    """constant A -> float32 [2 variants, 2 k-chunks, 128, 512] moving ops."""
    AT = A.T.copy()
    out = np.empty((2, 2, 128, 512), np.float32)
    for k in range(2):
        r = AT.real[k * 128:(k + 1) * 128, :]
        i = AT.imag[k * 128:(k + 1) * 128, :]
        out[0, k, :, 0:256] = r
        out[0, k, :, 256:512] = i
        out[1, k, :, 0:256] = -i
        out[1, k, :, 256:512] = r
    return out


_CONST_CACHE = {}


def _get_device_arrays():
    """Host constants packed into the device DMA layouts."""
    if "dev" not in _CONST_CACHE:
        W1, mats = _host_constants()
        # moving constants [128, 6*2048]: col = a*2048 + (v*2+k)*512 + n
        movA = np.concatenate(
            [_pack_moving(A).reshape(4, 128, 512).transpose(1, 0, 2).reshape(128, 2048)
             for A in mats], axis=1)
        # w1 table [48, 128, 1024] d-major over padded depths
        w1rows = []
        for d in range(NDP):
            dd = d if d < ND else 0
            for c in range(NB):
                w1rows.append(_pack_field(W1[dd, c]).transpose(1, 0, 2).reshape(128, 1024))
        w1all = np.stack(w1rows)
        R = np.kron(np.eye(16), np.ones((1, 16))).astype(np.float32)
        _CONST_CACHE["dev"] = (np.ascontiguousarray(movA),
                               np.ascontiguousarray(w1all), R)
    return _CONST_CACHE["dev"]


# ---------------------------------------------------------------- device program
_REPS = int(os.environ.get("BASS_KERNEL_REPS", "1"))


def _build_program():
    host_arrays = _get_device_arrays()
    reps = _REPS
    import concourse.bass as bass
    import concourse.bass_isa as bass_isa
    import concourse.bacc as bacc
    import concourse.mybir as mybir
    import concourse.tile as tile
    from collections import deque

    dt = mybir.dt
    ALU = mybir.AluOpType
    ACTF = mybir.ActivationFunctionType

    movA_h, w1all_h, R_h = host_arrays

    nc = bacc.Bacc("TRN2", target_bir_lowering=False, debug=False,
                   num_devices=NCORES)

    def inline(data, name, f32r=False):
        h = nc.inline_tensor(np.ascontiguousarray(data), name=name)
        if f32r:
            mls = nc.lookup_mls(h)
            mls.dtype = dt.float32r
            h = bass.DRamTensorHandle(name, list(data.shape), dt.float32r)
        return h.ap()

    movA_d = inline(movA_h, "mova", f32r=True)                 # [128, 12288]
    w1all_d = inline(w1all_h, "w1all", f32r=True)              # [48, 128, 1024]
    r_d = inline(R_h, "rmat")                                  # [16, 256]

    img_d = nc.dram_tensor("imgf", [128, 6144], dt.float32r, kind="ExternalInput").ap()
    map_d = nc.dram_tensor("mapf", [B, 128, DPC * 512], dt.float32, kind="ExternalInput").ap()
    ht_d = nc.dram_tensor("ht", [16, 16], dt.float32, kind="ExternalInput").ap()
    par_d = nc.dram_tensor("param", [1, 1], dt.float32, kind="ExternalInput").ap()
    out_d = nc.dram_tensor("out_recov", [DPC, NB, B, 128, 512], dt.float32, kind="ExternalOutput").ap()
    outm_d = nc.dram_tensor("out_max", [1, DPC], dt.float32, kind="ExternalOutput").ap()

    with tile.TileContext(nc) as tc:
        with (
            tc.tile_pool(name="res", bufs=1) as res,
            tc.tile_pool(name="wk", bufs=2) as wk,
            tc.tile_pool(name="ps", bufs=4, space="PSUM") as ps,
            tc.tile_pool(name="dram", bufs=1, space="DRAM") as dram,
        ):
            # ---------------- resident constants (per-matrix DMAs)
            movall = res.tile([128, 6 * 2048], dt.float32r, tag="movall", name="movall")
            for a in range(6):
                nc.sync.dma_start(movall[:, a * 2048:(a + 1) * 2048],
                                  movA_d[:, a * 2048:(a + 1) * 2048])

            def mov(a, v, k):
                o = a * 2048 + (v * 2 + k) * 512
                return movall[:, o:o + 512]

            par1 = res.tile([1, 1], dt.float32, tag="par1", name="par1")
            nc.sync.dma_start(par1[:], par_d[:])
            par128 = res.tile([128, 1], dt.float32, tag="par128", name="par128")
            nc.gpsimd.partition_broadcast(par128[:], par1[:])

            # ---------------- CA = R^T @ (H @ R)  (plain fp32)
            ht_t = res.tile([16, 16], dt.float32, tag="ht_t", name="ht_t")
            r_t = res.tile([16, 256], dt.float32, tag="r_t", name="r_t")
            nc.sync.dma_start(ht_t[:], ht_d[:])
            nc.sync.dma_start(r_t[:], r_d[:])
            ca_mid_ps = ps.tile([16, 256], dt.float32, tag="psB", bufs=4, name="ca_mid_ps")
            nc.tensor.matmul(ca_mid_ps[:], ht_t[:], r_t[:], start=True, stop=True)
            ca_mid = res.tile([16, 256], dt.float32, tag="ca_mid", name="ca_mid")
            nc.vector.tensor_copy(ca_mid[:], ca_mid_ps[:])
            # ca2 [128,1024] = [rb0: ca|ca, rb1: ca|ca] for one-op ph multiply
            ca2 = res.tile([128, 1024], dt.float32r, tag="ca2", name="ca2")
            for mb in range(2):
                ca_ps = ps.tile([128, 256], dt.float32, tag="psB", bufs=4, name=f"ca_ps{mb}")
                nc.tensor.matmul(ca_ps[:], r_t[:, mb * 128:(mb + 1) * 128],
                                 ca_mid[:], start=True, stop=True)
                nc.vector.tensor_copy(ca2[:, mb * 512:mb * 512 + 256], ca_ps[:])
                nc.scalar.copy(ca2[:, mb * 512 + 256:mb * 512 + 512], ca_ps[:])

            # ---------------- helpers
            def mm_half(stat, a_idx, is_complex, ptag, name):
                """PSUM[mb][128,512] = S^T @ A^T.  `stat` = 2 per-k-chunk APs:
                complex: [128,512] ([Re|Im]); real: [128,256]."""
                psums = []
                for mb in range(2):
                    acc = ps.tile([128, 512], dt.float32, tag=ptag, bufs=4,
                                  name=f"{name}_ps{mb}")
                    mms = []
                    for k in range(2):
                        mms.append((stat[k][:, mb * 128:(mb + 1) * 128], mov(a_idx, 0, k)))
                        if is_complex:
                            mms.append((stat[k][:, 256 + mb * 128:256 + (mb + 1) * 128],
                                        mov(a_idx, 1, k)))
                    for i, (lhsT, rhs) in enumerate(mms):
                        nc.tensor.matmul(acc[:], lhsT, rhs,
                                         start=(i == 0), stop=(i == len(mms) - 1))
                    psums.append(acc)
                return psums

            def mm_real_out(stat, a_idx, name):
                """One PSUM [128,512]: cols mb*256 hold Re(S^T A^T) row-block mb."""
                acc = ps.tile([128, 512], dt.float32, tag="psB", bufs=4, name=f"{name}_ps")
                for mb in range(2):
                    mms = []
                    for k in range(2):
                        mms.append((stat[k][:, mb * 128:(mb + 1) * 128],
                                    mov(a_idx, 0, k)[:, 0:256]))
                        mms.append((stat[k][:, 256 + mb * 128:256 + (mb + 1) * 128],
                                    mov(a_idx, 1, k)[:, 0:256]))
                    for i, (lhsT, rhs) in enumerate(mms):
                        nc.tensor.matmul(acc[:, mb * 256:(mb + 1) * 256], lhsT, rhs,
                                         start=(i == 0), stop=(i == len(mms) - 1))
                return acc

            def drain2(psums, name):
                out = [wk.tile([128, 512], dt.float32r, tag="drA", bufs=8,
                               name=f"{name}{mb}") for mb in range(2)]
                nc.scalar.copy(out[0][:], psums[0][:])
                nc.vector.tensor_copy(out[1][:], psums[1][:])
                return out

            def view3(ap_tile, lo):
                """[128,1024] tile -> strided [128,2,256] view of Re (lo=0) or Im."""
                return ap_tile[:].rearrange("p (b x) -> p b x", b=2)[:, :, lo:lo + 256]

            def cmul_fused(out_t, x_t, y_t):
                """out = x * y for [128,1024] rb-packed complex tiles.
                4 mults on Pool, combine on DVE (strided rb-fused views)."""
                xr, xi = view3(x_t, 0), view3(x_t, 256)
                yr, yi = view3(y_t, 0), view3(y_t, 256)
                t = [wk.tile([128, 512], dt.float32, tag="cmt", bufs=4, name=f"cmt{i}")
                     for i in range(4)]
                tv = [q[:].rearrange("p (b x) -> p b x", b=2) for q in t]
                nc.gpsimd.tensor_tensor(tv[0], xr, yr, op=ALU.mult)
                nc.gpsimd.tensor_tensor(tv[1], xi, yi, op=ALU.mult)
                nc.gpsimd.tensor_tensor(tv[2], xr, yi, op=ALU.mult)
                nc.gpsimd.tensor_tensor(tv[3], xi, yr, op=ALU.mult)
                nc.vector.tensor_tensor(view3(out_t, 0), tv[0], tv[1], op=ALU.subtract)
                nc.vector.tensor_tensor(view3(out_t, 256), tv[2], tv[3], op=ALU.add)

            def pump(gens, depth):
                q = deque()
                it = iter(gens)
                while True:
                    while len(q) < depth:
                        g = next(it, None)
                        if g is None:
                            break
                        q.append(g)
                    if not q:
                        break
                    g = q.popleft()
                    try:
                        next(g)
                        q.append(g)
                    except StopIteration:
                        pass

            # ---------------- resident per-unit products
            psffr = [res.tile([128, 1024], dt.float32, tag=f"psffr{i}", name=f"psffr{i}")
                     for i in range(DPC * NB)]
            runmax = [res.tile([128, 1], dt.float32, tag=f"runmax{dl}", name=f"runmax{dl}")
                      for dl in range(DPC)]

            imgft_dr = dram.tile([B * NB, 128, 1024], dt.float32, name="imgft_dr")
            kker_dr = dram.tile([DPC * NB, 128, 1024], dt.float32, name="kker_dr")

            pid6 = nc.gpsimd.partition_id() * (DPC * NB)

            for _rep in range(reps):
                cc_in = [dram.tile([NB, 128, 512], dt.float32, name=f"cc_in{b}_r{_rep}")
                         for b in range(B)]
                cc_out = [dram.tile([NB, 128, 512], dt.float32, name=f"cc_out{b}_r{_rep}",
                                    addr_space="Shared") for b in range(B)]

                # ======== stage 1: imgft + psf/psffr/K units, pipelined
                def imgft_gen(f):
                    imS = wk.tile([128, 512], dt.float32r, tag="imS", bufs=3, name="imS")
                    nc.sync.dma_start(imS[:], img_d[:, f * 512:(f + 1) * 512])
                    pa = mm_half([imS[:, 0:256], imS[:, 256:512]], FC_I, False,
                                 "psA", f"ifa{f}")
                    yield
                    u1 = drain2(pa, f"ifu{f}")
                    pb = mm_half(u1, FC_I, True, "psB", f"ifb{f}")
                    yield
                    imo = wk.tile([128, 1024], dt.float32, tag="cfld", bufs=3, name="imo")
                    nc.scalar.copy(imo[:, 0:512], pb[0][:])
                    nc.vector.tensor_copy(imo[:, 512:1024], pb[1][:])
                    nc.scalar.dma_start(imgft_dr[f], imo[:])

                def psf_gen(u):
                    c = u % NB
                    w1t = wk.tile([128, 1024], dt.float32r, tag="w1t", bufs=2, name="w1t")
                    nc.gpsimd.dma_start(w1t[:], w1all_d[bass.ds(pid6 + u, 1)])
                    # ph = w1 * CA, in place (f32r bits == f32)
                    nc.gpsimd.tensor_tensor(w1t[:], w1t[:], ca2[:], op=ALU.mult)
                    p1 = mm_half([w1t[:, 0:512], w1t[:, 512:1024]], A1_I, True,
                                 "psA", "s1a")
                    yield
                    u1 = drain2(p1, "s1u1")
                    p2 = mm_half(u1, A1_I, True, "psB", "s1b")
                    yield
                    u2 = drain2(p2, "s1u2")
                    p3 = mm_half(u2, A2C_I + c, True, "psA", "s1c")
                    yield
                    u3 = drain2(p3, "s1u3")
                    p4 = mm_half(u3, A2C_I + c, True, "psB", "s1d")
                    yield
                    # psf = |vu|^2 (unnormalized); row-sums via activation accum
                    sq = [wk.tile([128, 512], dt.float32r, tag="drA", bufs=8,
                                  name=f"sq{rb}") for rb in range(2)]
                    acc = [wk.tile([128, 1], dt.float32, tag="tiny", bufs=24,
                                   name=f"acc{rb}") for rb in range(2)]
                    for rb in range(2):
                        nc.scalar.activation(sq[rb][:], p4[rb][:],
                                             ACTF.Square, accum_out=acc[rb][:])
                    psft = wk.tile([128, 512], dt.float32r, tag="psft", bufs=3, name="psft")
                    for rb in range(2):
                        nc.vector.tensor_tensor(psft[:, rb * 256:(rb + 1) * 256],
                                                sq[rb][:, 0:256], sq[rb][:, 256:512],
                                                op=ALU.add)
                    tot = wk.tile([128, 1], dt.float32, tag="tiny", bufs=24, name="tot")
                    nc.vector.tensor_tensor(tot[:], acc[0][:], acc[1][:], op=ALU.add)
                    tot128 = wk.tile([128, 1], dt.float32, tag="tot128", bufs=6, name="tot128")
                    nc.gpsimd.partition_all_reduce(tot128[:], tot[:], channels=128,
                                                   reduce_op=bass_isa.ReduceOp.add)
                    inv128 = wk.tile([128, 1], dt.float32, tag="tiny", bufs=24, name="inv128")
                    nc.vector.reciprocal(inv128[:], tot128[:])
                    stp = [psft[:, 0:256], psft[:, 256:512]]
                    f1 = mm_half(stp, FC_I, False, "psA", "pfa")
                    yield
                    fu = drain2(f1, "pfu")
                    f2 = mm_half(fu, FC_I, True, "psB", "pfb")
                    yield
                    # psffr = (Fc psf Fc)/sum  -- scale rides the drain
                    nc.scalar.activation(psffr[u][:, 0:512], f2[0][:], ACTF.Copy,
                                         scale=inv128[:])
                    nc.vector.tensor_scalar_mul(psffr[u][:, 512:1024], f2[1][:], inv128[:])
                    g1 = mm_half(stp, GC_I, False, "psA", "pia")
                    yield
                    gu = drain2(g1, "piu")
                    g2 = mm_half(gu, GC_I, True, "psB", "pib")
                    yield
                    # K = (Gc psf Gc) / ((|psffr|^2 + p) * sum)
                    kk = wk.tile([128, 1024], dt.float32, tag="cfld", bufs=3, name="kk")
                    for rb in range(2):
                        sqk = wk.tile([128, 512], dt.float32, tag="magb", bufs=3, name="sqk")
                        nc.scalar.activation(sqk[:], psffr[u][:, rb * 512:(rb + 1) * 512],
                                             ACTF.Square)
                        den = wk.tile([128, 256], dt.float32, tag="den", bufs=2, name="den")
                        nc.vector.scalar_tensor_tensor(den[:], sqk[:, 0:256], par128[:],
                                                       sqk[:, 256:512],
                                                       op0=ALU.add, op1=ALU.add)
                        nc.vector.tensor_scalar_mul(den[:], den[:], tot128[:])
                        inv = wk.tile([128, 256], dt.float32, tag="den", bufs=2, name="invd")
                        nc.vector.reciprocal(inv[:], den[:])
                        nc.vector.tensor_tensor(kk[:, rb * 512:rb * 512 + 256],
                                                g2[rb][:, 0:256], inv[:], op=ALU.mult)
                        nc.vector.tensor_tensor(kk[:, rb * 512 + 256:(rb + 1) * 512],
                                                g2[rb][:, 256:512], inv[:], op=ALU.mult)
                    nc.scalar.dma_start(kker_dr[u], kk[:])

                s1gens = []
                npsf = 0
                for f in range(B * NB):
                    s1gens.append(imgft_gen(f))
                    if f % 2 == 1 and npsf < DPC * NB:
                        s1gens.append(psf_gen(npsf))
                        npsf += 1
                pump(s1gens, 4)

                # ======== blur stage: result(b,c) = sum_dl Map*|Gc(imf*psffr)Gc|
                imf_t = {}
                maps_t = {}
                racc_t = {}

                def blur_gen(b, dl, c):
                    if c == 0:
                        maps_t[(b, dl)] = wk.tile([128, 512], dt.float32, tag="maps",
                                                  bufs=2, name="maps")
                        nc.sync.dma_start(maps_t[(b, dl)][:],
                                          map_d[b][:, dl * 512:(dl + 1) * 512])
                    if dl == 0:
                        imf_t[(b, c)] = wk.tile([128, 1024], dt.float32, tag="imf",
                                                bufs=3, name="imf")
                        nc.sync.dma_start(imf_t[(b, c)][:], imgft_dr[b * NB + c])
                    bp = wk.tile([128, 1024], dt.float32r, tag="cprod", bufs=2, name="bp")
                    cmul_fused(bp, imf_t[(b, c)], psffr[dl * NB + c])
                    p1 = mm_half([bp[:, 0:512], bp[:, 512:1024]], GC_I, True, "psA", "bla")
                    yield
                    u1 = drain2(p1, "blu")
                    p2 = mm_real_out(u1, GC_I, "blb")
                    yield
                    mag = wk.tile([128, 512], dt.float32, tag="magb", bufs=3, name="mag")
                    nc.scalar.activation(mag[:], p2[:], ACTF.Abs)
                    ms = maps_t[(b, dl)][:]
                    if dl == 0:
                        racc_t[(b, c)] = wk.tile([128, 512], dt.float32, tag="racc",
                                                 bufs=3, name="racc")
                        nc.gpsimd.tensor_tensor(racc_t[(b, c)][:], mag[:], ms, op=ALU.mult)
                    else:
                        t = wk.tile([128, 512], dt.float32, tag="magb", bufs=3, name="rt")
                        nc.gpsimd.tensor_tensor(t[:], mag[:], ms, op=ALU.mult)
                        nc.gpsimd.tensor_tensor(racc_t[(b, c)][:], racc_t[(b, c)][:],
                                                t[:], op=ALU.add)
                        nc.sync.dma_start(cc_in[b][c], racc_t[(b, c)][:])
                        if c == NB - 1:
                            nc.gpsimd.collective_compute(
                                "AllReduce", ALU.add,
                                replica_groups=[list(range(NCORES))],
                                ins=[cc_in[b][:]], outs=[cc_out[b][:]],
                            )

                blgens = [blur_gen(b, dl, c)
                          for b in range(B) for dl in range(DPC) for c in range(NB)]
                pump(blgens, 4)

                # ======== wiener stage
                for dl in range(DPC):
                    nc.vector.memset(runmax[dl][:], 0.0)
                resfr_t = {}

                def rf_gen(b, c):
                    resfr_t[(b, c)] = wk.tile([128, 1024], dt.float32, tag="resfr",
                                              bufs=3, name="resfr")
                    rres = wk.tile([128, 512], dt.float32, tag="magb", bufs=3, name="rres")
                    nc.sync.dma_start(rres[:], cc_out[b][c])
                    res_t = wk.tile([128, 512], dt.float32r, tag="imS", bufs=3, name="res_t")
                    nc.scalar.copy(res_t[:], rres[:])
                    p1 = mm_half([res_t[:, 0:256], res_t[:, 256:512]], FC_I, False,
                                 "psA", "rfa")
                    yield
                    u1 = drain2(p1, "rfu")
                    p2 = mm_half(u1, FC_I, True, "psB", "rfb")
                    yield
                    nc.scalar.copy(resfr_t[(b, c)][:, 0:512], p2[0][:])
                    nc.vector.tensor_copy(resfr_t[(b, c)][:, 512:1024], p2[1][:])

                def wn_gen(b, c, dl):
                    kkt = wk.tile([128, 1024], dt.float32, tag="cfld", bufs=3, name="kkt")
                    nc.sync.dma_start(kkt[:], kker_dr[dl * NB + c])
                    wn = wk.tile([128, 1024], dt.float32r, tag="cprod", bufs=2, name="wn")
                    cmul_fused(wn, kkt, resfr_t[(b, c)])
                    p1 = mm_half([wn[:, 0:512], wn[:, 512:1024]], GC_I, True, "psA", "wna")
                    yield
                    u1 = drain2(p1, "wnu")
                    p2 = mm_real_out(u1, GC_I, "wnb")
                    yield
                    mag2 = wk.tile([128, 512], dt.float32, tag="fin", bufs=2, name="mag2")
                    nc.scalar.activation(mag2[:], p2[:], ACTF.Abs)
                    nc.scalar.dma_start(out_d[dl, c, b], mag2[:])
                    piece = wk.tile([128, 1], dt.float32, tag="tiny", bufs=24, name="piece")
                    nc.vector.tensor_reduce(piece[:], mag2[:], axis=mybir.AxisListType.X,
                                            op=ALU.max)
                    nc.vector.tensor_tensor(runmax[dl][:], runmax[dl][:], piece[:],
                                            op=ALU.max)

                wgens = []
                for b in range(B):
                    for c in range(NB):
                        wgens.append(rf_gen(b, c))
                    for c in range(NB):
                        for dl in range(DPC):
                            wgens.append(wn_gen(b, c, dl))
                pump(wgens, 3)

                # ======== per-core max -> tiny output (host divides)
                outm = wk.tile([1, DPC], dt.float32, tag="outm", bufs=2, name="outm")
                for dl in range(DPC):
                    gmx = wk.tile([128, 1], dt.float32, tag="tiny", bufs=24, name="gmx")
                    nc.gpsimd.partition_all_reduce(gmx[:], runmax[dl][:], channels=128,
                                                   reduce_op=bass_isa.ReduceOp.max)
                    nc.vector.tensor_copy(outm[0:1, dl:dl + 1], gmx[0:1, :])
                nc.sync.dma_start(outm_d[:], outm[:])

    nc.compile()
    return nc


_PROG_CACHE = {}


def _get_program():
    if "nc" not in _PROG_CACHE:
        _PROG_CACHE["nc"] = _build_program()
    return _PROG_CACHE["nc"]


# ---------------------------------------------------------------- cached runner
def _make_runner():
    """Build the jitted SPMD callable once; reuse across kernel() calls."""
    import jax
    from jax.sharding import Mesh, PartitionSpec
    from jax.experimental.shard_map import shard_map
    import concourse.mybir as mybir
    from concourse import bass2jax

    bass2jax.install_neuronx_cc_hook()
    nc = _get_program()

    partition_name = nc.partition_id_tensor.name if nc.partition_id_tensor else None
    in_names, out_names, out_avals, zero_shapes = [], [], [], []
    for alloc in nc.m.functions[0].allocations:
        if not isinstance(alloc, mybir.MemoryLocationSet):
            continue
        if not alloc.memorylocations:
            continue
        name = alloc.memorylocations[0].name
        if alloc.kind == "ExternalInput":
            if name != partition_name:
                in_names.append(name)
        elif alloc.kind == "ExternalOutput":
            out_names.append(name)
            shape = tuple(alloc.tensor_shape)
            dtype = mybir.dt.np(alloc.dtype)
            out_avals.append(jax.core.ShapedArray(shape, dtype))
            zero_shapes.append((shape, dtype))
    n_params = len(in_names)
    n_outs = len(out_avals)
    all_in_names = list(in_names) + list(out_names)
    if partition_name is not None:
        all_in_names.append(partition_name)
    donate = tuple(range(n_params, n_params + n_outs))

    def _body(*args):
        operands = list(args)
        if partition_name is not None:
            operands.append(bass2jax.partition_id_tensor())
        outs = bass2jax._bass_exec_p.bind(
            *operands,
            out_avals=tuple(out_avals),
            in_names=tuple(all_in_names),
            out_names=tuple(out_names),
            lowering_input_output_aliases=(),
            sim_require_finite=True,
            sim_require_nnan=True,
            nc=nc,
        )
        return tuple(outs)

    devices = jax.devices()[:NCORES]
    mesh = Mesh(np.asarray(devices), ("core",))
    in_specs = (PartitionSpec("core"),) * (n_params + n_outs)
    out_specs = (PartitionSpec("core"),) * n_outs
    sharded = jax.jit(
        shard_map(_body, mesh=mesh, in_specs=in_specs, out_specs=out_specs,
                  check_rep=False),
        donate_argnums=donate, keep_unused=True)

    def run(in_maps):
        concat_in = [
            np.concatenate([np.asarray(m[name]) for m in in_maps], axis=0)
            for name in in_names
        ]
        concat_zeros = [
            np.zeros((NCORES * s[0], *s[1:]), d) for (s, d) in zero_shapes
        ]
        out_arrs = sharded(*concat_in, *concat_zeros)
        return [
            {name: np.asarray(out_arrs[i]).reshape(NCORES, *out_avals[i].shape)[c]
             for i, name in enumerate(out_names)}
            for c in range(NCORES)
        ]

    return run


def _get_runner():
    if "run" not in _PROG_CACHE:
        _PROG_CACHE["run"] = _make_runner()
    return _PROG_CACHE["run"]


# ---------------------------------------------------------------- entry point
def _build_in_maps(img, Map, H, parameter):
    # img fields (b,c) -> [128, 6144]: col = (f*2+k)*256 + x
    imgt = img.transpose(0, 3, 1, 2).reshape(B * NB, 2, 128, 256)
    imgf = np.ascontiguousarray(imgt.transpose(2, 0, 1, 3).reshape(128, B * NB * 512))
    # Map -> per-core [4, 128, DPC*512]: col = (dl*2+rb)*256 + x
    mapt = Map.transpose(3, 0, 1, 2).reshape(ND, B, 2, 128, 256)
    ht = np.ascontiguousarray(H.reshape(16, 16).T)
    par = parameter.reshape(1, 1)
    in_maps = []
    for core in range(NCORES):
        mp = np.zeros((B, 128, DPC * 512), np.float32)
        for dl in range(DPC):
            d = core * DPC + dl
            if d < ND:
                fld = mapt[d].transpose(0, 2, 1, 3).reshape(B, 128, 512)
                mp[:, :, dl * 512:(dl + 1) * 512] = fld
        in_maps.append({
            "imgf": imgf, "mapf": mp, "ht": ht, "param": par,
        })
    return in_maps


def kernel(img, Map, H, parameter):
    img = np.ascontiguousarray(np.asarray(img, np.float32))
    Map = np.ascontiguousarray(np.asarray(Map, np.float32))
    H = np.asarray(H, np.float32)
    parameter = np.asarray(parameter, np.float32)

    try:
        run = _get_runner()
    except Exception:
        run = None

    in_maps = _build_in_maps(img, Map, H, parameter)

    if run is not None:
        try:
            results = run(in_maps)
        except Exception:
            run = None
    if run is None:
        from concourse.bass_utils import run_bass_kernel_spmd
        rr = run_bass_kernel_spmd(_get_program(), in_maps,
                                  core_ids=list(range(NCORES)))
        results = rr.results

    gmax = 0.0
    for core in range(NCORES):
        om = results[core]["out_max"].reshape(DPC)
        for dl in range(DPC):
            if core * DPC + dl < ND:
                gmax = max(gmax, float(om[dl]))
    inv = 1.0 / gmax if gmax > 0 else 1.0

    out = np.empty((B, 256, 256, NB * ND), np.float32)
    for core in range(NCORES):
        rec = results[core]["out_recov"]            # [DPC, NB, B, 128, 512]
        for dl in range(DPC):
            d = core * DPC + dl
            if d >= ND:
                continue
            for c in range(NB):
                for b in range(B):
                    out[b, :, :, c * ND + d] = (
                        rec[dl, c, b].reshape(128, 2, 256)
                        .transpose(1, 0, 2).reshape(256, 256))
    out *= inv
    return out
